# revision 8
# baseline (speedup 1.0000x reference)
"""Depthwise 3D transposed conv (stride 2, k=4, SAME) on 8 trn2 NeuronCores.

x: (4, 32, 32, 32, 256) f32, filters: (4, 4, 4, 1, 256) f32
y: (4, 64, 64, 64, 256) f32

Sharding: 8 cores = (batch n in 4) x (d-halves 2). Zero communication.

Compute structure (v2): h-taps folded into the matmul contraction.
Stationary S[(j,hi,cc) 96, (r,ph,ho4,c') 128] per (g 32, pw 2, dwi 2)
has 4 nonzeros/column (2 kd x 2 kh) = 512 useful MACs/cycle; the two
w-taps (dwi) accumulate in PSUM via w-shifted rhs windows; rhs free =
(k2 2, blk 8, b 32) = 512 (two plane-pair tiles per matmul). f16 stores.

v4 scheduling fixes (from the v3 trace):
 - duplicated planes are re-read from HBM at full DMA width (the
   48-partition SBUF->SBUF dedup copies were slower than HBM re-reads
   and stretched matmul pacing via SBUF port contention)
 - half-pair slabs with bufs=5 break the store->evac->PSUM->PE
   backpressure chain
 - k2-major store layout keeps every store contiguous per partition
 - pair 0 loads all three planes straight from HBM (no copy on the
   startup critical path); first weight chunk is 8 matrices, not 32
"""
import sys

sys.path.insert(0, "/opt/trn_rl_repo")

from contextlib import ExitStack

import numpy as np

import concourse.bass as bass  # noqa: F401  (registers engine classes)
import concourse.tile as tile
from concourse import bacc, mybir
from concourse.bass_utils import run_bass_kernel_spmd

F32 = mybir.dt.float32
F16 = mybir.dt.float16

N_CORES = 8
TAPS = {0: [(-1, 3), (0, 1)], 1: [(0, 2), (1, 0)]}
KD = {0: (2, 0), 1: (3, 1)}  # KD[r][j]
NK = 17  # plane-pair tiles; tile t holds planes (t, t+1)
NP = 9  # tile-pairs: pair p covers tiles (2p, 2p+1); tile 17 is dropped
WCHUNKS = [(0, 8), (8, 32), (32, 64), (64, 96), (96, 128)]

_PROG = None


def _build_program():
    nc = bacc.Bacc(
        "TRN2", target_bir_lowering=False, debug=False, num_devices=N_CORES
    )
    # xp: one copy of each plane: [gh, plane q, (hi,cc) 48, gl, blk, w]
    xp_d = nc.declare_dram_parameter("xp", [2, 18, 48, 16, 8, 34], F16, isOutput=False)
    # wtab: [(j,hi,cc), m=(g,pw,dwi), (r,ph,ho4,c')]
    wt_d = nc.declare_dram_parameter("wtab", [96, 128, 128], F16, isOutput=False)
    # y: [pair, gh, glh, q=(r,ph,ho4,c'), k2, gl, pw, blk, b]
    y_d = nc.declare_dram_parameter(
        "y", [NP, 2, 2, 128, 2, 8, 2, 8, 32], F16, isOutput=True
    )

    with ExitStack() as ctx:
        tc = ctx.enter_context(tile.TileContext(nc))
        wpool = ctx.enter_context(tc.tile_pool(name="wpool", bufs=1))
        xpool = ctx.enter_context(tc.tile_pool(name="xpool", bufs=4))
        spool = ctx.enter_context(tc.tile_pool(name="spool", bufs=6))
        ppool = ctx.enter_context(tc.tile_pool(name="ppool", bufs=4, space="PSUM"))

        wt = wpool.tile([96, 128, 128], F16)
        wt_loaded = set()

        def load_wchunk_i(ci):
            if 0 <= ci < len(WCHUNKS) and ci not in wt_loaded:
                m0, m1 = WCHUNKS[ci]
                nc.sync.dma_start(out=wt[:, m0:m1, :], in_=wt_d[:, m0:m1, :])
                wt_loaded.add(ci)

        def load_wchunk(g):
            # safety net; chunks are normally prefetched by issue_load
            for ci, (m0, m1) in enumerate(WCHUNKS):
                if m0 <= g * 4 < m1:
                    load_wchunk_i(ci)

        SEQ = [(gh, p) for gh in range(2) for p in range(NP)]
        tiles = {}

        def issue_load(i):
            # explicit prefetch: tile loads issue 3 pairs ahead of use so
            # the gh transition and thin last pair never starve the PE
            if i >= len(SEQ):
                return
            gh, p = SEQ[i]
            xt = xpool.tile([96, 2, 16, 8, 34], F16, tag="xt")
            nk2 = 1 if p == NP - 1 else 2
            # full-width plane-pair loads; duplicated planes re-read
            # from HBM (48-partition SBUF copies measured ~2x slower
            # per byte and stretched matmul pacing via port contention)
            srcs = [
                xp_d[gh, 2 * p + k2 : 2 * p + k2 + 2].rearrange(
                    "p a gl blk w -> (p a) gl blk w"
                )
                for k2 in range(nk2)
            ]
            if i == 0:
                # first pair: land gl 0-7 of BOTH k2 halves first (the
                # first matmuls need both), then the first weight chunk,
                # then the gl 8-15 halves
                for k2 in range(nk2):
                    nc.sync.dma_start(out=xt[:, k2, 0:8], in_=srcs[k2][:, 0:8])
                load_wchunk_i(0)
                for k2 in range(nk2):
                    nc.sync.dma_start(out=xt[:, k2, 8:16], in_=srcs[k2][:, 8:16])
            elif i <= 2:
                # early pairs: smaller load quanta while the matmul
                # stream is still chasing the first transfers
                for k2 in range(nk2):
                    for glh2 in range(2):
                        sl = slice(glh2 * 8, glh2 * 8 + 8)
                        nc.sync.dma_start(out=xt[:, k2, sl], in_=srcs[k2][:, sl])
            else:
                for k2 in range(nk2):
                    nc.sync.dma_start(out=xt[:, k2], in_=srcs[k2])
            tiles[i] = xt
            # interleave the next weight chunk behind this tile's loads so
            # all of wtab lands in the first ~20us without ever queuing a
            # needed chunk behind multiple 1.7MB tile transfers
            load_wchunk_i(i + 1)

        for i in range(3):
            issue_load(i)
        evac_i = 0
        for i, (gh, p) in enumerate(SEQ):
            if True:
                xt = tiles.pop(i)
                nk2 = 1 if p == NP - 1 else 2

                for glh in range(2):
                    slab = spool.tile([128, 2, 8, 2, 8, 32], F16, tag="slab")
                    for gl8 in range(8):
                        gl = glh * 8 + gl8
                        g = gh * 16 + gl
                        load_wchunk(g)
                        ps = ppool.tile([128, 2, 2, 8, 32], F32, tag="ps", name="ps")
                        for pw in range(2):
                            for dwi in range(2):
                                m = (g * 2 + pw) * 2 + dwi
                                dw = TAPS[pw][dwi][0]
                                nc.tensor.matmul(
                                    ps[:, pw, 0:nk2],
                                    wt[:, m, :],
                                    xt[:, 0:nk2, gl, :, 1 + dw : 33 + dw],
                                    start=(dwi == 0),
                                    stop=(dwi == 1),
                                )
                        out_ap = slab[:, :, gl8].rearrange(
                            "q k2 pw blk b -> q pw k2 blk b"
                        )
                        if evac_i % 2:
                            nc.vector.tensor_copy(out_ap, ps[:])
                        else:
                            nc.scalar.copy(out_ap, ps[:])
                        evac_i += 1
                    # SWDGE stores; skip out-of-range boundary slots (p=0:
                    # r=0 of tile 0 is plane -1; p=8: only tile 16 r=0)
                    if p == 0:
                        nc.gpsimd.dma_start(
                            out=y_d[0, gh, glh, 64:128], in_=slab[64:128]
                        )
                        nc.gpsimd.dma_start(
                            out=y_d[0, gh, glh, 0:64, 1], in_=slab[0:64, 1]
                        )
                    elif p == NP - 1:
                        nc.gpsimd.dma_start(
                            out=y_d[p, gh, glh, 0:64, 0], in_=slab[0:64, 0]
                        )
                    else:
                        nc.gpsimd.dma_start(out=y_d[p, gh, glh], in_=slab[:])
                issue_load(i + 3)
    nc.compile()
    return nc


def _get_program():
    global _PROG
    if _PROG is None:
        _PROG = _build_program()
    return _PROG


def _make_wtab(filters):
    ftap = np.asarray(filters, np.float32)[:, :, :, 0, :]  # (kd, kh, kw, c)
    wtab = np.zeros((96, 128, 128), np.float16)
    idx = np.arange(8)
    for g in range(32):
        for pw in range(2):
            for dwi in range(2):
                m = (g * 2 + pw) * 2 + dwi
                kw = TAPS[pw][dwi][1]
                for r in range(2):
                    for j in range(2):
                        kd = KD[r][j]
                        for ph in range(2):
                            for dh, kh in TAPS[ph]:
                                for ho4 in range(4):
                                    hi = ho4 + dh + 1
                                    wtab[
                                        j * 48 + hi * 8 + idx,
                                        m,
                                        r * 64 + ph * 32 + ho4 * 8 + idx,
                                    ] = ftap[kd, kh, kw, g * 8 + idx]
    return wtab


def _make_in_maps(x, filters):
    from numpy.lib.stride_tricks import sliding_window_view

    x = np.asarray(x, np.float32)
    wtab = _make_wtab(filters)

    in_maps = []
    for core in range(N_CORES):
        n, hf = core // 2, core % 2
        lo = 16 * hf - 1
        planes = np.zeros((18, 32, 32, 256), np.float32)
        s0, s1 = max(lo, 0), min(16 * hf + 17, 32)
        planes[s0 - lo : s1 - lo] = x[n, s0:s1]
        planes = planes.transpose(0, 3, 1, 2)  # (18, c, h, w)
        padded = np.zeros((18, 256, 34, 34), np.float16)
        padded[:, :, 1:33, 1:33] = planes  # pad index = coord + 1
        pg = padded.reshape(18, 32, 8, 34, 34)  # (plane, g, cc, H, w)
        # h-blocks: H = blk*4 + hi, hi in 0..5 -> overlapping 6-row windows
        sw = sliding_window_view(pg, 6, axis=3)[:, :, :, ::4]  # (18,32,8,8,34,6)
        a = sw.transpose(0, 5, 2, 1, 3, 4)  # (plane, hi, cc, g, blk, w)
        xp = a.reshape(18, 48, 2, 16, 8, 34).transpose(2, 0, 1, 3, 4, 5)
        in_maps.append({"xp": np.ascontiguousarray(xp), "wtab": wtab})
    return in_maps


def kernel(x, filters):
    nc = _get_program()
    in_maps = _make_in_maps(x, filters)
    res = run_bass_kernel_spmd(nc, in_maps, list(range(N_CORES)))
    y = np.empty((4, 64, 64, 64, 256), np.float32)
    for core in range(N_CORES):
        n, hf = core // 2, core % 2
        yc = res.results[core]["y"]  # [pair, gh, glh, q, k2, gl8, pw, blk, b]
        yc = yc.reshape(NP, 2, 2, 2, 2, 4, 8, 2, 8, 2, 8, 32)
        # dims: pair0 gh1 glh2 r3 ph4 ho4_5 cp6 k2_7 gl8_8 pw9 blk10 b11
        yt = yc.transpose(0, 7, 3, 10, 5, 4, 11, 9, 1, 2, 8, 6)
        # (pair, k2, r, blk, ho4, ph, b, pw, gh, glh, gl8, cp)
        yt = yt.reshape(36, 64, 64, 256)[1:33]
        y[n, 32 * hf : 32 * hf + 32] = yt.astype(np.float32)
    return y
